# revision 12
# baseline (speedup 1.0000x reference)
"""ConsensusAttention Trainium2 kernel (v2).

Shapes (hardcoded): levels [B=8, N=1024, L=6, D=128] fp32.
Sharding: batch b across the 8 cores (data parallel); each core runs all
L=6 "heads" for its batch.

Math per (b, l):
  q = x, k = x / ||x||, sim[i, j] = (q_i . k_j) / sqrt(D)
  sim[i, i] = -0.0005 ; sim[i, j] = -inf where grid_dist(i, j) > 2
  out = softmax_j(sim) @ x

Structure (see v1 docstring for the banded-mask derivation):
  * 32x32-grid radius-2 mask => scores banded (|i-j| <= 64); each 128-row
    j-block of the transposed score matrix S'[j, i] only needs a 256-col
    i-window.  E tiles live in per-jb 384-wide frames [z64|win256|z64] so
    attn@V reads aligned 128-col chunks (zero strips cover the overhang).
  * Scores for a j-block PAIR share one 2KB PSUM bank; ONE identity-lhsT
    matmul per pair adds the (-60000 masked / 0) bias for both halves.
  * rs[j] = 1/(sqrt(D)*||x_j||) is computed entirely on the DVE via the
    Quake rsqrt bit-trick + 2 Newton iterations (fp32) - no Ln activation,
    so the ACT engine only ever loads the Exp table (once).
  * attn@V: lhsT = E chunks, rhs = [V | 1] (ones column -> denominator in
    the same PSUM tile).  Self-attention diagonal rides as a c0*I matmul.
  * o-accumulators packed 3 per PSUM bank; normalize = one tensor_scalar
    divide per block (numerator / denominator, per-partition scalar).
  * All matmul operands fp16 (1 col/cycle); fp32 PSUM accumulation.
  * All DMAs issue from the SP queue (HWDGE) - keeps ACT/DVE sequencers
    free; head-0 input load is split in halves to cut the cold start.
"""

from contextlib import ExitStack

import numpy as np

import concourse.bacc as bacc
import concourse.tile as tile
from concourse import mybir
from concourse.alu_op_type import AluOpType
from concourse.bass_utils import run_bass_kernel_spmd

B, N, L, D = 8, 1024, 6, 128
NB = N // 128  # 8 token blocks of 128
NP = NB // 2  # 4 j-block pairs
GRID = 32
RADIUS = 2.0
SELF_VAL = -0.0005
F32 = mybir.dt.float32
F16 = mybir.dt.float16
I32 = mybir.dt.int32
RSQRT_MAGIC = 0x5F3759DF
INV_SQRT_D = float(D) ** -0.5


def _win(jb: int) -> int:
    """Start of the 256-col score window for j-block jb (covers the
    |i-j|<=64 band; clipped at the edges)."""
    return min(max(jb * 128 - 64, 0), N - 256)


def _contrib(jb: int):
    """Output blocks that j-block jb contributes to."""
    return [ib for ib in (jb - 1, jb, jb + 1) if 0 <= ib < NB]


def _build_constants():
    yy, xx = np.meshgrid(np.arange(GRID), np.arange(GRID), indexing="ij")
    coors = np.stack([yy.ravel(), xx.ravel()], axis=-1).astype(np.float32)
    dist = np.sqrt(((coors[:, None, :] - coors[None, :, :]) ** 2).sum(-1))
    bad = (dist > np.float32(RADIUS)) | np.eye(N, dtype=bool)  # [j, i] masked

    # mb[p] = mask bias for the paired score bank of j-blocks (2p, 2p+1):
    # cols [0,256) mask block 2p's window, [256,512) block 2p+1's.
    mb = np.empty((NP, 128, 512), np.float16)
    for jb in range(NB):
        w0 = _win(jb)
        half = (jb % 2) * 256
        mb[jb // 2, :, half : half + 256] = np.where(
            bad[jb * 128 : (jb + 1) * 128, w0 : w0 + 256], -60000.0, 0.0
        ).astype(np.float16)
        # Every allowed (j, i) pair must fall inside the window, and every
        # aligned-chunk overhang outside the window must be fully masked
        # (those E positions are the frame's zero strips).
        assert bad[jb * 128 : (jb + 1) * 128, :w0].all()
        assert bad[jb * 128 : (jb + 1) * 128, w0 + 256 :].all()
        for ib in _contrib(jb):
            off = ib * 128 - w0  # chunk start relative to window
            assert -64 <= off <= 192, (jb, ib, off)

    ident = np.eye(128, dtype=np.float16)
    c0i = (np.exp(np.float32(SELF_VAL)) * np.eye(128)).astype(np.float16)
    return mb, np.stack([ident, c0i])


def _emit(tc: tile.TileContext, ctx: ExitStack, xh, mb, cns, out):
    nc = tc.nc
    const = ctx.enter_context(tc.tile_pool(name="const", bufs=1))
    xin = ctx.enter_context(tc.tile_pool(name="xin", bufs=1))
    xtp = ctx.enter_context(tc.tile_pool(name="xtp", bufs=2))
    small = ctx.enter_context(tc.tile_pool(name="small", bufs=4))
    scr = ctx.enter_context(tc.tile_pool(name="scr", bufs=2))
    stg = ctx.enter_context(tc.tile_pool(name="stg", bufs=2))
    tp = ctx.enter_context(tc.tile_pool(name="tp", bufs=2, space="PSUM"))
    sp = ctx.enter_context(tc.tile_pool(name="sp", bufs=3, space="PSUM"))
    op = ctx.enter_context(tc.tile_pool(name="op", bufs=3, space="PSUM"))

    xh_v = xh.rearrange("(b p) l d -> p b l d", p=128)
    out_v = out.rearrange("(b p) l d -> p b l d", p=128)
    mb_v = mb.rearrange("j p c -> p j c")

    # --- input DMAs: x loads on the SP HWDGE queue; constants + mask ride
    # the ACT queue (idle early) so head-0 data leads on SP.
    cns_v = cns.rearrange("k p d -> p k d")
    cns_sb = const.tile([128, 2, 128], F16, name="cns_sb")
    ident = cns_sb[:, 0, :]
    c0ih = cns_sb[:, 1, :]
    nc.scalar.dma_start(out=cns_sb, in_=cns_v)
    xh_all = xin.tile([128, NB, L, D + 1], F16, name="xh_all")
    nc.sync.dma_start(out=xh_all[:, 0:4, 0, 0:D], in_=xh_v[:, 0:4, 0, :])
    nc.sync.dma_start(out=xh_all[:, 4:8, 0, 0:D], in_=xh_v[:, 4:8, 0, :])
    mb_sb = const.tile([128, NP, 512], F16, name="mb_sb")
    nc.scalar.dma_start(out=mb_sb, in_=mb_v)
    for l in range(1, L):
        nc.sync.dma_start(out=xh_all[:, :, l, 0:D], in_=xh_v[:, :, l, :])
    nc.vector.memset(xh_all[:, :, :, D : D + 1], 1.0)

    # E frames: [128, 2, 384] per j-block pair; each subframe is
    # [z64 | win 256 | z64].  Zero strips memset ONCE (gpsimd - idle engine)
    # and never rewritten; tiles rotate manually so strips stay valid.
    e_tiles = []
    for k in range(3):
        t = const.tile([128, 2, 384], F16, tag=f"e{k}", name=f"e{k}")
        nc.gpsimd.memset(t[:, :, 0:64], 0.0)
        nc.gpsimd.memset(t[:, :, 320:384], 0.0)
        e_tiles.append(t)

    # norm2 kept in fp16 so the DVE reduce runs in 2x packed mode (the
    # ~5e-4 relative rounding on ||x||^2 is far inside the error budget);
    # each NR chain upcasts its slice to fp32 first.
    norm2 = small.tile([128, L, NB], F16, name="norm2")
    rs = small.tile([128, L, NB], F32, name="rs")

    def emit_norms(l, blo, bhi):
        # sum_d x^2 per token: fp16 squares in DVE 2x packed mode; the
        # (expensive, ~1us) reduction runs on the otherwise-idle Pool engine
        # for heads >= 1 (head 0's reduce is cold-start critical -> DVE).
        sq = scr.tile([128, NB, D], F16, tag="sq", name=f"sq_{l}_{blo}")
        mul = nc.vector if l == 0 else nc.gpsimd
        mul.tensor_mul(
            sq[:, blo:bhi],
            xh_all[:, blo:bhi, l, 0:D],
            xh_all[:, blo:bhi, l, 0:D],
        )
        with nc.allow_low_precision(reason="fp16 norm2: ~5e-4 rel, gate 2e-2"):
            nc.vector.reduce_sum(
                norm2[:, l, blo:bhi], sq[:, blo:bhi], axis=mybir.AxisListType.X
            )

    def emit_rs(llo, lhi):
        # rs = (1/sqrt(D)) * rsqrt(norm2), Quake seed + 2 Newton steps.
        # All fp32 on the DVE; rel err ~4e-6 - well inside the fp16 noise.
        g = lhi - llo
        a = small.tile([128, g, NB], F32, tag="nr_a", name=f"a_{llo}")
        nc.vector.tensor_copy(out=a, in_=norm2[:, llo:lhi, :])
        y = small.tile([128, g, NB], F32, tag="nr_y", name=f"y_{llo}")
        t = small.tile([128, g, NB], F32, tag="nr_t", name=f"t_{llo}")
        # seed: y = bitcast(MAGIC - (bits(a) >> 1)) == bitcast(-(bits>>1)*1 + MAGIC)
        nc.vector.tensor_scalar(
            y.bitcast(I32),
            a.bitcast(I32),
            1,
            None,
            op0=AluOpType.logical_shift_right,
        )
        nc.vector.tensor_scalar(
            y.bitcast(I32), y.bitcast(I32), -1, RSQRT_MAGIC,
            op0=AluOpType.mult, op1=AluOpType.add,
        )
        for last in (False, True):
            nc.vector.tensor_mul(t, y, y)  # y^2
            nc.vector.tensor_mul(t, t, a)  # a*y^2
            # w = 1.5 - 0.5*a*y^2  (fold 1/sqrt(D) into the last step)
            c = INV_SQRT_D if last else 1.0
            nc.vector.tensor_scalar(
                t, t, -0.5 * c, 1.5 * c, op0=AluOpType.mult, op1=AluOpType.add
            )
            nc.vector.tensor_mul(rs[:, llo:lhi, :] if last else y, y, t)

    def emit_transposes(l, half, pt, xt):
        # XT[d, token] via fp16 PE transposes; one 512-col half at a time so
        # head 0 can start on its first DMA half.
        for b in range(4 * half, 4 * half + 4):
            nc.tensor.matmul(
                pt[:, b * 128 : (b + 1) * 128],
                lhsT=xh_all[:, b, l, 0:D],
                rhs=ident,
                is_transpose=True,
                start=(b % 4 == 0),
                stop=(b % 4 == 3),
            )
        nc.vector.tensor_copy(
            out=xt[:, half * 512 : (half + 1) * 512],
            in_=pt[:, half * 512 : (half + 1) * 512],
        )

    def new_xt(l):
        pt = tp.tile([128, N], F16, tag="pt", name=f"pt_{l}")
        xt = xtp.tile([128, N], F16, tag="xt", name=f"xt_{l}")
        return pt, xt

    # Head 0 front matter: transposes + norms as the two DMA halves land.
    pt0, xt0 = new_xt(0)
    emit_transposes(0, 0, pt0, xt0)
    emit_norms(0, 0, 4)
    emit_transposes(0, 1, pt0, xt0)
    emit_norms(0, 4, 8)
    emit_rs(0, 1)

    xt_cur = xt0
    ei = 0  # rotating E-frame index

    for l in range(L):
        xt = xt_cur

        def scores(p, e):
            # Two 256-col score matmuls into one PSUM pair bank + ONE
            # mask-bias matmul for the whole pair, then per-half Exp with
            # rs as the per-partition ACT scale.
            s_ps = sp.tile([128, 2, 256], F32, tag="s", name=f"s_{l}_{p}")
            for h in range(2):
                jb = 2 * p + h
                w0 = _win(jb)
                nc.tensor.matmul(
                    s_ps[:, h, :],
                    lhsT=xt[:, jb * 128 : (jb + 1) * 128],
                    rhs=xt[:, w0 : w0 + 256],
                    start=(h == 0),
                    stop=False,
                    skip_group_check=True,
                )
            nc.tensor.matmul(
                s_ps[:, :, :],
                lhsT=ident,
                rhs=mb_sb[:, p, :],
                start=False,
                stop=True,
                skip_group_check=True,
            )
            for h in range(2):
                jb = 2 * p + h
                nc.scalar.activation(
                    e[:, h, 64:320],
                    s_ps[:, h, :],
                    mybir.ActivationFunctionType.Exp,
                    scale=rs[:, l, jb : jb + 1],
                )

        o_banks = {}
        closed = {}

        def attnv(jb, e):
            h = jb % 2
            w0 = _win(jb)
            for ib in _contrib(jb):
                off = 64 + ib * 128 - w0  # chunk start in the 384 frame
                k, slot = divmod(ib, 3)
                first = jb == max(ib - 1, 0)
                last = jb == min(ib + 1, NB - 1)
                if first:
                    bank_start = k not in o_banks
                    if bank_start:
                        o_banks[k] = op.tile(
                            [128, 3, D + 1], F32, tag="o", name=f"o_{l}_{k}"
                        )
                        closed[k] = 0
                    nc.tensor.matmul(
                        o_banks[k][:, slot, :],
                        lhsT=c0ih,
                        rhs=xh_all[:, ib, l, :],
                        start=bank_start,
                        stop=False,
                        skip_group_check=True,
                    )
                nc.tensor.matmul(
                    o_banks[k][:, slot, :],
                    lhsT=e[:, h, off : off + 128],
                    rhs=xh_all[:, jb, l, :],
                    start=False,
                    stop=last,
                    skip_group_check=True,
                )
                if last:
                    closed[k] += 1
                    nblk = 2 if k == 2 else 3
                    if closed[k] == nblk:
                        ob = o_banks.pop(k)
                        rcp = small.tile(
                            [128, nblk, 1], F32, tag="rcp", name=f"rcp_{l}_{k}"
                        )
                        nc.vector.reciprocal(rcp[:, :, 0], ob[:, 0:nblk, D])
                        # one broadcast multiply per bank (GPSIMD cannot
                        # touch PSUM, so these all stay on the DVE)
                        nc.vector.tensor_mul(
                            stage[:, k * 3 : k * 3 + nblk, :],
                            ob[:, 0:nblk, 0:D],
                            rcp.broadcast_to([128, nblk, D]),
                        )
                        bank_done(k)

        stage = stg.tile([128, NB, D], F32, tag="stage", name=f"stage_{l}")

        def bank_done(k):
            if l == L - 1:
                if k == 1:
                    nc.sync.dma_start(
                        out=out_v[:, 0:6, l, :], in_=stage[:, 0:6, :]
                    )
                elif k == 2:
                    nc.sync.dma_start(
                        out=out_v[:, 6:8, l, :], in_=stage[:, 6:8, :]
                    )

        e_p = []
        for p in range(NP):
            e = e_tiles[ei % 3]
            ei += 1
            e_p.append(e)
            scores(p, e)
            if p == 1:
                # Hide exp latency: head l+1's transposes + next heads' norms
                # (rs for head l+1 must be ready before head l+1's first exp).
                if l + 1 < L:
                    pt_n, xt_n = new_xt(l + 1)
                    emit_transposes(l + 1, 0, pt_n, xt_n)
                    emit_transposes(l + 1, 1, pt_n, xt_n)
                    xt_cur = xt_n
                if l == 0:
                    emit_norms(1, 0, 8)
                    emit_rs(1, 2)
                elif l == 1:
                    emit_norms(2, 0, 8)
                    emit_norms(3, 0, 8)
                    emit_rs(2, 4)
                elif l == 2:
                    emit_norms(4, 0, 8)
                    emit_norms(5, 0, 8)
                    emit_rs(4, 6)
                attnv(0, e_p[0])
                attnv(1, e_p[0])
            elif p == 2:
                attnv(2, e_p[1])
                attnv(3, e_p[1])
            elif p == 3:
                attnv(4, e_p[2])
                attnv(5, e_p[2])
        attnv(6, e_p[3])
        attnv(7, e_p[3])
        assert not o_banks
        if l < L - 1:
            nc.sync.dma_start(out=out_v[:, :, l, :], in_=stage)


def build_nc():
    nc = bacc.Bacc("TRN2", target_bir_lowering=False, debug=False, num_devices=B)
    xh = nc.dram_tensor("xh", [N, L, D], F16, kind="ExternalInput").ap()
    mb = nc.dram_tensor("mb", [NP, 128, 512], F16, kind="ExternalInput").ap()
    cns = nc.dram_tensor("cns", [2, 128, 128], F16, kind="ExternalInput").ap()
    out = nc.dram_tensor("out", [N, L, D], F32, kind="ExternalOutput").ap()
    with tile.TileContext(nc) as tc:
        with ExitStack() as ctx:
            _emit(tc, ctx, xh, mb, cns, out)
    nc.compile()
    return nc


_NC = None


def _get_nc():
    global _NC
    if _NC is None:
        _NC = build_nc()
    return _NC


def run_spmd(levels: np.ndarray, trace: bool = False):
    """Run on the 8 NeuronCores; returns (out [B,N,L,D], exec_time_ns|None)."""
    levels = np.ascontiguousarray(levels, dtype=np.float32)
    assert levels.shape == (B, N, L, D), levels.shape
    mb, cns = _build_constants()
    nc = _get_nc()
    xh = levels.astype(np.float16)
    in_maps = [{"xh": xh[b], "mb": mb, "cns": cns} for b in range(B)]
    res = run_bass_kernel_spmd(
        nc, in_maps, core_ids=list(range(B)), trace=trace
    )
    out = np.stack([res.results[b]["out"] for b in range(B)]).astype(np.float32)
    return out, res.exec_time_ns


def kernel(levels: np.ndarray) -> np.ndarray:
    out, _ = run_spmd(levels, trace=False)
    return out
